# revision 17
# baseline (speedup 1.0000x reference)
"""Trainium2 Bass kernel for nn_BEM_50002009260181 — pair-split softmax.

Same math as kernel.py, but each core computes scores ONLY for its own
L-half (4 l-tiles); the softmax denominator is completed by exchanging
the per-half exp-sums (2 floats) between the two cores of a batch with a
tiny AllGather over core pairs.  This halves the score-phase tanh/matmul
/reduce work, which is the ACT-engine bottleneck.

ACT order: tanh (keys + taq/tvq + vval, exp_and_others set) -> exp ->
[AllGather z] -> Sin x8 (one table switch), with the unnormalized
AValue product (sn/cos * recip) precomputed on DVE so only 8 cheap bf16
tensor_scalar ops + output DMAs remain after the collective lands.
"""

import numpy as np

B, L, D, F = 4, 1024, 768, 32
NCORES = 8
LT = 128
NT_OWN = 4         # own-half l-tiles (512 rows)
LH = NT_OWN * LT   # 512
K1 = F + 1
VOFF = 64

PI = float(np.pi)
PIO2_HI = float(np.float32(np.pi / 2))

_CACHE = {}


def _build():
    if "nc" in _CACHE:
        return _CACHE["nc"]

    import concourse.bacc as bacc
    from concourse import bass_isa
    import concourse.tile as tile
    import concourse.mybir as mybir
    from concourse.tile import add_dep_helper

    F32 = mybir.dt.float32
    F32R = mybir.dt.float32r
    BF16 = mybir.dt.bfloat16
    AF = mybir.ActivationFunctionType
    ALU = mybir.AluOpType

    nc = bacc.Bacc(num_devices=NCORES)

    def mmr(out, lhsT, rhs):
        nc.tensor.matmul(out, lhsT, rhs, start=True, stop=True)

    # ---- DRAM I/O (per-core shapes; own L-half only) ----
    d_t = nc.dram_tensor("t_rot", [LH, D], F32, kind="ExternalInput")
    d_av = nc.dram_tensor("av_pack", [VOFF + K1, LH], F32R, kind="ExternalInput")
    d_rhs = nc.dram_tensor("rhs_pack", [VOFF + K1, 2 * D], F32R, kind="ExternalInput")
    d_wq = nc.dram_tensor("w_q", [VOFF + F, 1], F32R, kind="ExternalInput")
    d_b = nc.dram_tensor("b_ab", [LT, 2], F32, kind="ExternalInput")
    # full-precision copies for the tan path (fp32r DMA rounding would be
    # amplified through 1/cos near the tan poles)
    d_ava = nc.dram_tensor("av_a32", [K1, LH], F32, kind="ExternalInput")
    d_rhsa = nc.dram_tensor("rhs_a32", [K1, D], F32, kind="ExternalInput")
    d_oa = nc.dram_tensor("out_a", [LH, D], BF16, kind="ExternalOutput")
    d_ov = nc.dram_tensor("out_v", [LH, D], BF16, kind="ExternalOutput")
    # z exchange buffers
    d_zout = nc.dram_tensor("z_part", [1, 2], F32, kind="Internal")
    d_zall = nc.dram_tensor("z_all", [1, 4], F32, kind="Internal")

    t_view = d_t.rearrange("(n p) d -> p n d", p=LT)     # [128, 4, 768]
    oa_view = d_oa.rearrange("(n p) d -> p n d", p=LT)
    ov_view = d_ov.rearrange("(n p) d -> p n d", p=LT)

    with tile.TileContext(nc) as tc:
        with (
            tc.tile_pool(name="consts", bufs=1) as consts,
            tc.tile_pool(name="keys", bufs=2) as keys,
            tc.tile_pool(name="vals", bufs=1) as vals,
            tc.tile_pool(name="vwork", bufs=3) as vwork,
            tc.tile_pool(name="ps", bufs=1, space="PSUM") as ps,
        ):
            # ---- inputs into SBUF; smallest startup-critical slices first ----
            sb_av = consts.tile([VOFF + K1, LH], F32R, tag="sb_av")
            nc.sync.dma_start(out=sb_av[0:K1, 0:LT], in_=d_av[0:K1, 0:LT])
            nc.sync.dma_start(out=sb_av[VOFF : VOFF + K1, 0:LT], in_=d_av[VOFF : VOFF + K1, 0:LT])
            sb_rhs = consts.tile([VOFF + K1, 2 * D], F32R, tag="sb_rhs")
            nc.sync.dma_start(out=sb_rhs[0:K1, 0:D], in_=d_rhs[0:K1, 0:D])
            nc.sync.dma_start(out=sb_rhs[VOFF : VOFF + K1, 0:D], in_=d_rhs[VOFF : VOFF + K1, 0:D])
            sb_wq = consts.tile([VOFF + F, 1], F32R, tag="sb_wq")
            nc.sync.dma_start(out=sb_wq[:], in_=d_wq[:])
            sb_b = consts.tile([LT, 2], F32, tag="sb_b")
            nc.sync.dma_start(out=sb_b[:], in_=d_b[:])
            # memsets first: the Pool engine is in-order, and the PE warmup
            # needs dmy0 before the Q7 descriptor-gens monopolize the engine
            sb_hi = consts.tile([LT, 1], F32, tag="sb_hi")
            nc.gpsimd.memset(sb_hi[:], PIO2_HI)
            nc.gpsimd.dma_start(out=sb_av[0:K1, LT:LH], in_=d_av[0:K1, LT:LH])
            nc.gpsimd.dma_start(out=sb_av[VOFF : VOFF + K1, LT:LH], in_=d_av[VOFF : VOFF + K1, LT:LH])
            t_all = consts.tile([LT, NT_OWN, D], F32, tag="t_all")
            nc.gpsimd.dma_start(out=t_all[:, 0:1, :], in_=t_view[:, 0:1, :])
            nc.gpsimd.dma_start(out=t_all[:, 1:NT_OWN, :], in_=t_view[:, 1:NT_OWN, :])
            nc.gpsimd.dma_start(out=sb_rhs[VOFF : VOFF + K1, D : 2 * D], in_=d_rhs[VOFF : VOFF + K1, D : 2 * D])

            sb_ava = consts.tile([K1, LH], F32, tag="sb_ava")
            nc.gpsimd.dma_start(out=sb_ava[:], in_=d_ava[:])
            sb_rhsa = consts.tile([K1, D], F32, tag="sb_rhsa")
            nc.gpsimd.dma_start(out=sb_rhsa[:], in_=d_rhsa[:])
            A0, A1 = 0, K1
            V0, V1 = VOFF, VOFF + K1

            def emit_keys(ps_ak, ps_vk, lsl):
                mmr(ps_ak[:, 0:512], sb_av[A0:A1, lsl], sb_rhs[A0:A1, 0:512])
                mmr(ps_ak[:, 512:D], sb_av[A0:A1, lsl], sb_rhs[A0:A1, 512:D])
                mmr(ps_vk[:, 0:512], sb_av[V0:V1, lsl], sb_rhs[V0:V1, 0:512])
                mmr(ps_vk[:, 512:D], sb_av[V0:V1, lsl], sb_rhs[V0:V1, 512:D])

            ps_ak0 = ps.tile([LT, D], F32, tag="ak", name="ps_ak0")
            ps_vk0 = ps.tile([LT, D], F32, tag="vk", name="ps_vk0")
            emit_keys(ps_ak0, ps_vk0, slice(0, LT))

            # ---- qa/qv per-l scalars ----
            ps_q = ps.tile([LT, 2 * NT_OWN], F32, tag="val", bufs=2)
            for i in range(NT_OWN):
                nc.tensor.matmul(
                    ps_q[:, 2 * i : 2 * i + 1],
                    sb_av[0:F, i * LT : (i + 1) * LT].bitcast(F32),
                    sb_wq[0:F, :].bitcast(F32),
                    start=True, stop=True,
                )
                nc.tensor.matmul(
                    ps_q[:, 2 * i + 1 : 2 * i + 2],
                    sb_av[VOFF : VOFF + F, i * LT : (i + 1) * LT].bitcast(F32),
                    sb_wq[VOFF : VOFF + F, :].bitcast(F32),
                    start=True, stop=True,
                )
            sb_q = consts.tile([LT, 2 * NT_OWN], F32, tag="sb_q")
            nc.vector.tensor_copy(out=sb_q[:], in_=ps_q[:])

            s_ta = consts.tile([LT, NT_OWN], F32, tag="s_ta")
            s_tv = consts.tile([LT, NT_OWN], F32, tag="s_tv")
            out_v_sb = consts.tile([LT, NT_OWN, D], BF16, tag="out_v_sb")
            out_a_sb = consts.tile([LT, NT_OWN, D], BF16, tag="out_a_sb")
            vvals, rss, naxs = [], [], []

            def emit_xa(j):
                lsl = slice(j * LT, (j + 1) * LT)
                ps_xa = ps.tile([LT, D], F32, tag="val", bufs=2, name=f"ps_xa{j}")
                mmr(ps_xa[:, 0:512], sb_ava[:, lsl], sb_rhsa[:, 0:512])
                mmr(ps_xa[:, 512:D], sb_ava[:, lsl], sb_rhsa[:, 512:D])
                rs = vals.tile([LT, D], F32, tag=f"rs{j}", name=f"rs{j}")
                nc.vector.add_range_wrap(out=rs[:], in_=ps_xa[:], shift=0.0, bound=PI, period=2 * PI)
                nax = vals.tile([LT, D], F32, tag=f"nax{j}", name=f"nax{j}")
                nc.vector.scalar_tensor_tensor(
                    out=nax[:], in0=rs[:], scalar=-1.0, in1=rs[:],
                    op0=ALU.mult, op1=ALU.min,
                )
                rss.append(rs)
                naxs.append(nax)

            vval_insts = []

            def emit_xv(j):
                lsl = slice(j * LT, (j + 1) * LT)
                ps_xv = ps.tile([LT, D], F32, tag="val", bufs=2, name=f"ps_xv{j}")
                mmr(ps_xv[:, 0:512], sb_av[V0:V1, lsl], sb_rhs[V0:V1, D : D + 512])
                mmr(ps_xv[:, 512:D], sb_av[V0:V1, lsl], sb_rhs[V0:V1, D + 512 : 2 * D])
                vval = vals.tile([LT, D], BF16, tag=f"vval{j}", name=f"vval{j}")
                vval_insts.append(nc.scalar.activation(out=vval[:], in_=ps_xv[:], func=AF.Tanh))
                vvals.append(vval)

            # ---- score phase over own half ----
            for i in range(NT_OWN):
                lsl = slice(i * LT, (i + 1) * LT)
                if i == 0:
                    ps_ak, ps_vk = ps_ak0, ps_vk0
                else:
                    ps_ak = ps.tile([LT, D], F32, tag="ak", name=f"ps_ak{i}")
                    ps_vk = ps.tile([LT, D], F32, tag="vk", name=f"ps_vk{i}")
                    emit_keys(ps_ak, ps_vk, lsl)
                akey = keys.tile([LT, D], F32, tag="akey")
                nc.scalar.activation(out=akey[:], in_=ps_ak[:], func=AF.Tanh)
                taq = keys.tile([LT, D], F32, tag="taq")
                nc.scalar.activation(out=taq[:], in_=t_all[:, i, :], func=AF.Tanh,
                                     bias=sb_b[:, 0:1], scale=sb_q[:, 2 * i : 2 * i + 1])
                vkey = keys.tile([LT, D], F32, tag="vkey")
                nc.scalar.activation(out=vkey[:], in_=ps_vk[:], func=AF.Tanh)
                tvq = keys.tile([LT, D], F32, tag="tvq")
                nc.scalar.activation(out=tvq[:], in_=t_all[:, i, :], func=AF.Tanh,
                                     bias=sb_b[:, 1:2], scale=sb_q[:, 2 * i + 1 : 2 * i + 2])

                scr = keys.tile([LT, D], F32, tag="scr")
                nc.vector.scalar_tensor_tensor(
                    out=scr[:], in0=taq[:], scalar=1.0, in1=vkey[:],
                    op0=ALU.mult, op1=ALU.mult, accum_out=s_ta[:, i : i + 1],
                )
                scr2 = keys.tile([LT, D], F32, tag="scr2")
                nc.vector.scalar_tensor_tensor(
                    out=scr2[:], in0=tvq[:], scalar=1.0, in1=akey[:],
                    op0=ALU.mult, op1=ALU.mult, accum_out=s_tv[:, i : i + 1],
                )
                # weave value-phase A-side work in (rs/nax drain on DVE)
                if i >= 1:
                    emit_xa(i - 1)

            emit_xa(3)

            # ---- value phase V-side first (psum ring order: xv before ps_z;
            # vval tanh rides the tanh table set before exp) ----
            for j in range(NT_OWN):
                emit_xv(j)

            # ---- exp + z partial + pair AllGather ----
            e_ta = consts.tile([LT, NT_OWN], F32, tag="e_ta")
            e_tv = consts.tile([LT, NT_OWN], F32, tag="e_tv")
            rsum = consts.tile([LT, 2], F32, tag="rsum")
            nc.scalar.activation(out=e_ta[:], in_=s_ta[:], func=AF.Exp, accum_out=rsum[:, 0:1])
            exp_inst = nc.scalar.activation(out=e_tv[:], in_=s_tv[:], func=AF.Exp, accum_out=rsum[:, 1:2])
            zsum = consts.tile([LT, 2], F32, tag="zsum")
            nc.gpsimd.partition_all_reduce(zsum[:], rsum[:], channels=LT,
                                           reduce_op=bass_isa.ReduceOp.add)
            zdma = nc.sync.dma_start(out=d_zout[:], in_=zsum[0:1, :])
            cc = nc.gpsimd.collective_compute(
                "AllGather", ALU.bypass,
                replica_groups=[[0, 1], [2, 3], [4, 5], [6, 7]],
                ins=[d_zout[:].opt()], outs=[d_zall[:].opt()],
            )
            add_dep_helper(cc.ins, zdma.ins, sync=True, reason="cc after z write")
            # broadcast-read the gathered z pair to every partition in one
            # DMA (stride-0 partition view) so the whole combine stays on DVE
            z4b = consts.tile([LT, 4], F32, tag="z4b")
            zback = nc.sync.dma_start(out=z4b[:], in_=d_zall[:].partition_broadcast(LT))
            add_dep_helper(zback.ins, cc.ins, sync=True, reason="z read after cc")
            z2b = consts.tile([LT, 2], F32, tag="z2b")
            nc.vector.tensor_tensor(out=z2b[:], in0=z4b[:, 0:2], in1=z4b[:, 2:4], op=ALU.add)
            invzb = consts.tile([LT, 2], F32, tag="invzb")
            nc.vector.reciprocal(out=invzb[:], in_=z2b[:])

            # ---- sin/cos + exp-weighted AValue product (pa = e_ta*sin/cos),
            # accumulated straight into the contiguous out_a_sb staging tile so
            # the post-collective normalize is ONE wide 4x-mode op ----
            for j in range(NT_OWN):
                cs = vwork.tile([LT, D], F32, tag="cs", bufs=4)
                i2 = nc.scalar.activation(out=cs[:], in_=naxs[j][:], func=AF.Sin,
                                          bias=sb_hi[:])
                sn = vwork.tile([LT, D], F32, tag="sn", bufs=4)
                i1 = nc.scalar.activation(out=sn[:], in_=rss[j][:], func=AF.Sin)
                add_dep_helper(i1.ins, exp_inst.ins, sync=False, reason="sin after exp (ACT table set)")
                add_dep_helper(i2.ins, exp_inst.ins, sync=False, reason="sin after exp (ACT table set)")
                rc = vwork.tile([LT, D], F32, tag="rc")
                nc.vector.reciprocal_approx_fast(out=rc[:], in_=cs[:])
                nc.vector.scalar_tensor_tensor(
                    out=out_a_sb[:, j, :], in0=sn[:], scalar=e_ta[:, j : j + 1], in1=rc[:],
                    op0=ALU.mult, op1=ALU.mult,
                )
                nc.vector.tensor_scalar(out=out_v_sb[:, j, :], in0=vvals[j][:],
                                        scalar1=e_tv[:, j : j + 1], scalar2=None, op0=ALU.mult)

            # ---- post-collective: 2-tile-wide 4x normalizes, each DMA'd as
            # soon as its chunk is ready so the two 786KB output transfers
            # pipeline on the DMA engines instead of serializing at the end ----
            out_a2 = consts.tile([LT, NT_OWN, D], BF16, tag="out_a2")
            out_v2 = consts.tile([LT, NT_OWN, D], BF16, tag="out_v2")
            for c0 in range(0, NT_OWN, 2):
                nc.vector.tensor_scalar(out=out_v2[:, c0 : c0 + 2, :],
                                        in0=out_v_sb[:, c0 : c0 + 2, :],
                                        scalar1=invzb[:, 1:2], scalar2=None, op0=ALU.mult)
                nc.sync.dma_start(out=ov_view[:, c0 : c0 + 2, :], in_=out_v2[:, c0 : c0 + 2, :])
                nc.vector.tensor_scalar(out=out_a2[:, c0 : c0 + 2, :],
                                        in0=out_a_sb[:, c0 : c0 + 2, :],
                                        scalar1=invzb[:, 0:1], scalar2=None, op0=ALU.mult)
                nc.scalar.dma_start(out=oa_view[:, c0 : c0 + 2, :], in_=out_a2[:, c0 : c0 + 2, :])

    nc.finalize()
    _CACHE["nc"] = nc
    return nc


def _prep_in_maps(T, A, V, w_a, b_a, w_v, b_v,
                  W_aup1, b_aup1, W_aup2, b_aup2,
                  W_vup1, b_vup1, W_vup2, b_vup2):
    f32 = np.float32
    T = np.ascontiguousarray(np.asarray(T, f32))
    A = np.asarray(A, f32)
    V = np.asarray(V, f32)

    def aug_w(W, b):
        return np.concatenate([np.asarray(W, f32).T, np.asarray(b, f32)[None, :]], axis=0)

    rhs_pack = np.zeros((VOFF + K1, 2 * D), f32)
    rhs_pack[0:K1, 0:D] = aug_w(W_aup1, b_aup1)
    rhs_pack[0:K1, D : 2 * D] = aug_w(W_aup2, b_aup2)
    rhs_pack[VOFF : VOFF + K1, 0:D] = aug_w(W_vup1, b_vup1)
    rhs_pack[VOFF : VOFF + K1, D : 2 * D] = aug_w(W_vup2, b_vup2)

    w_q = np.zeros((VOFF + F, 1), f32)
    w_q[0:F, 0] = np.asarray(w_a, f32).reshape(F)
    w_q[VOFF : VOFF + F, 0] = np.asarray(w_v, f32).reshape(F)

    b_ab = np.empty((LT, 2), f32)
    b_ab[:, 0] = np.asarray(b_a, f32).reshape(())
    b_ab[:, 1] = np.asarray(b_v, f32).reshape(())

    in_maps = []
    for c in range(NCORES):
        b, h = divmod(c, 2)
        own = np.arange(512 * h, 512 * (h + 1))
        av_pack = np.zeros((VOFF + K1, LH), f32)
        av_pack[0:F] = A[b].T[:, own]
        av_pack[F] = 1.0
        av_pack[VOFF : VOFF + F] = V[b].T[:, own]
        av_pack[VOFF + F] = 1.0
        av_a32 = np.zeros((K1, LH), f32)
        av_a32[0:F] = A[b].T[:, own]
        av_a32[F] = 1.0
        in_maps.append({
            "t_rot": np.ascontiguousarray(T[b][own]),
            "av_pack": av_pack,
            "rhs_pack": rhs_pack,
            "w_q": w_q,
            "b_ab": b_ab,
            "av_a32": av_a32,
            "rhs_a32": np.ascontiguousarray(rhs_pack[0:K1, D : 2 * D]),
        })
    return in_maps


def kernel(**inputs):
    from concourse.bass_utils import run_bass_kernel_spmd

    nc = _build()
    in_maps = _prep_in_maps(**inputs)
    res = run_bass_kernel_spmd(nc, in_maps, core_ids=list(range(NCORES)))

    out_a = np.empty((B, L, D), np.float32)
    out_v = np.empty((B, L, D), np.float32)
    for c in range(NCORES):
        b, h = divmod(c, 2)
        out_a[b, 512 * h : 512 * (h + 1)] = np.asarray(res.results[c]["out_a"], np.float32)
        out_v[b, 512 * h : 512 * (h + 1)] = np.asarray(res.results[c]["out_v"], np.float32)
    return out_a, out_v
